# revision 11
# baseline (speedup 1.0000x reference)
"""Trainium2 Bass kernel for nn_Net_22840636080718 (gnn_message_passing).

The graph is circulant C_N(1, 97, 5001) with self loops. Nodes are re-indexed
by n = (2059*i) mod N so offsets become {+2059, -277, -2941} (max halo 2941);
all gathers/scatters become cyclic shifts = contiguous DMAs. Nodes shard
contiguously across 8 cores (12500 each); x lives in SBUF (feature-major
[128, chunk+2*halo]); e streams through DRAM per edge-type with per-type
halo-extended ranges so edge->node aggregation is core-local; one AllGather
of x boundary slabs per MP iteration, halo windows read back with
dynamic-slice DMAs (per-core row offsets from an input tensor).
Matmuls run in float32r (1 cyc/row at N>=256). LayerNorm uses host-centered
W2/b2 (mean-free outputs) + PE ones-reduce for variance + K=1 broadcast
matmul for the per-column inv-std*g scale. Final L post-processing
(reverse-mean, masks, L@x segment sums) is O(E) elementwise on host.
"""

import sys
import numpy as np

sys.path.insert(0, "/opt/trn_rl_repo")

import concourse.bass as bass
import concourse.bacc as bacc
import concourse.tile as tile
from concourse import mybir
from concourse.bass_utils import run_bass_kernel_spmd

F32 = mybir.dt.float32
F32R = mybir.dt.float32r
AF = mybir.ActivationFunctionType
ALU = mybir.AluOpType

N = 100000
NCORES = 8
M = N // NCORES          # 12500 nodes per core
H = 128                  # latent dim
NT = 512                 # tile width (matmul moving dim)
CMUL = 2059              # node re-index multiplier
N_ITERS = 30
IN_NODE = 6
IN_EDGE = 4
EPS = 1e-5

_NC_CACHE = {}


def _signed(o):
    o = o % N
    return o - N if o > N // 2 else o


def _plan(offs):
    # widths padded to even: fp32r matmuls require an even moving-dim size.
    halo = max(abs(o) for o in offs)
    xw = M + 2 * halo + 2
    eo, widths = [], []
    c = 0
    for o in offs:
        w = M + abs(o)
        w += w & 1
        eo.append(c)
        widths.append(w)
        c += w
    return halo, xw, eo, widths, c


def build_nc(offs, n_iters=N_ITERS):
    halo, xw, eo, widths, ew = _plan(offs)
    nmat = 9 * n_iters + 6
    nvec = 8 * n_iters + 12
    ngrow = 256 * n_iters + 256

    nc = bacc.Bacc("TRN2", target_bir_lowering=False)

    nattr = nc.dram_tensor("nattr", [IN_NODE, xw], F32R, kind="ExternalInput")
    eattr = nc.dram_tensor("eattr", [IN_EDGE, ew], F32R, kind="ExternalInput")
    wmats = nc.dram_tensor("wmats", [H, nmat * H], F32R, kind="ExternalInput")
    vecs = nc.dram_tensor("vecs", [H, nvec], F32, kind="ExternalInput")
    gvecs = nc.dram_tensor("gvecs", [1, ngrow], F32R, kind="ExternalInput")
    w0n = nc.dram_tensor("w0n", [IN_NODE, H], F32R, kind="ExternalInput")
    w0e = nc.dram_tensor("w0e", [IN_EDGE, H], F32R, kind="ExternalInput")
    wd2 = nc.dram_tensor("wd2", [H, 1], F32R, kind="ExternalInput")
    onesc = nc.dram_tensor("onesc", [H, 1], F32R, kind="ExternalInput")
    halo_off = nc.dram_tensor("halo_off", [1, 2], mybir.dt.uint32, kind="ExternalInput")
    l_out = nc.dram_tensor("L_out", [7, M], F32, kind="ExternalOutput")

    with tile.TileContext(nc) as tc:
        import contextlib
        with contextlib.ExitStack() as ctx:
            ctx.enter_context(nc.allow_low_precision(
                reason="float32r is full-width storage; matmul accumulates in fp32 PSUM"))
            persist = ctx.enter_context(tc.tile_pool(name="persist", bufs=1))
            wpool = ctx.enter_context(tc.tile_pool(name="wpool", bufs=2))
            sbt = ctx.enter_context(tc.tile_pool(name="sbt", bufs=3))
            rows = ctx.enter_context(tc.tile_pool(name="rows", bufs=2))
            psA = ctx.enter_context(tc.tile_pool(name="psA", bufs=2, space="PSUM"))
            psB = ctx.enter_context(tc.tile_pool(name="psB", bufs=2, space="PSUM"))
            psC = ctx.enter_context(tc.tile_pool(name="psC", bufs=2, space="PSUM"))
            psR = ctx.enter_context(tc.tile_pool(name="psR", bufs=2, space="PSUM"))
            dram = ctx.enter_context(tc.tile_pool(name="dram", bufs=1, space="DRAM"))

            x_ext = persist.tile([H, xw], F32R)
            agg = persist.tile([H, M], F32R)
            ones_col = persist.tile([H, 1], F32R)
            nc.sync.dma_start(out=ones_col, in_=onesc[:, :])

            e_loc = dram.tile([H, ew], F32R)
            ag_in = dram.tile([H, 2 * halo], F32R)

            wenc = persist.tile([H, 6 * H], F32R)
            nc.sync.dma_start(out=wenc, in_=wmats[:, 9 * n_iters * H:nmat * H])
            vecenc = persist.tile([H, 12], F32)
            nc.sync.dma_start(out=vecenc, in_=vecs[:, 8 * n_iters:nvec])
            genc = persist.tile([1, 256], F32R)
            nc.sync.dma_start(out=genc, in_=gvecs[:, 256 * n_iters:ngrow])
            w0n_sb = persist.tile([IN_NODE, H], F32R)
            nc.sync.dma_start(out=w0n_sb, in_=w0n[:, :])
            w0e_sb = persist.tile([IN_EDGE, H], F32R)
            nc.sync.dma_start(out=w0e_sb, in_=w0e[:, :])
            wd2_sb = persist.tile([H, 1], F32R)
            nc.sync.dma_start(out=wd2_sb, in_=wd2[:, :])

            rL = nc.gpsimd.alloc_register("rowL")
            nc.gpsimd.reg_load(rL, halo_off[0:1, 0:1])
            offL = nc.gpsimd.snap(rL)
            rR = nc.gpsimd.alloc_register("rowR")
            nc.gpsimd.reg_load(rR, halo_off[0:1, 1:2])
            offR = nc.gpsimd.snap(rR)

            def mm_acc(psum, pieces, nt):
                npc = len(pieces)
                for i, (lh, rh) in enumerate(pieces):
                    nc.tensor.matmul(out=psum[:, :nt], lhsT=lh, rhs=rh,
                                     start=(i == 0), stop=(i == npc - 1))

            def mlp_tile(nt, l0_pieces, w1_ap, w2_ap, b0_ap, b1_ap, bt2_ap,
                         beta_ap, g_ap, act, resid_ap, out_ap):
                """One [128, nt] tile of MLP + LayerNorm (+ optional residual).

                If out_ap is None, writes a fresh f32r tile (returned);
                otherwise writes out_ap and returns None.
                """
                h0_ps = psA.tile([H, NT], F32, tag="psA")
                mm_acc(h0_ps, l0_pieces, nt)
                h0 = sbt.tile([H, NT], F32R, tag="h0")
                nc.scalar.activation(out=h0[:, :nt], in_=h0_ps[:, :nt], func=act,
                                     bias=b0_ap)
                h1_ps = psB.tile([H, NT], F32, tag="psB")
                mm_acc(h1_ps, [(w1_ap, h0[:, :nt])], nt)
                h1 = sbt.tile([H, NT], F32R, tag="h1")
                nc.scalar.activation(out=h1[:, :nt], in_=h1_ps[:, :nt], func=act,
                                     bias=b1_ap)
                oc_ps = psC.tile([H, NT], F32, tag="psC")
                mm_acc(oc_ps, [(w2_ap, h1[:, :nt])], nt)
                oc = sbt.tile([H, NT], F32, tag="oc")
                nc.scalar.activation(out=oc[:, :nt], in_=oc_ps[:, :nt],
                                     func=AF.Identity, bias=bt2_ap)
                sq = sbt.tile([H, NT], F32R, tag="h1")
                nc.gpsimd.tensor_mul(sq[:, :nt], oc[:, :nt], oc[:, :nt])
                sq_ps = psR.tile([1, NT], F32, tag="psR")
                nc.tensor.matmul(out=sq_ps[:, :nt], lhsT=ones_col[:],
                                 rhs=sq[:, :nt], start=True, stop=True)
                sd = rows.tile([1, NT], F32, tag="sd")
                nc.scalar.activation(out=sd[:, :nt], in_=sq_ps[:, :nt],
                                     func=AF.Sqrt, bias=vecenc[0:1, 11:12], scale=1.0 / H)
                inv = rows.tile([1, NT], F32R, tag="inv")
                nc.vector.reciprocal(out=inv[:, :nt], in_=sd[:, :nt])
                a_ps = psA.tile([H, NT], F32, tag="psA")
                nc.tensor.matmul(out=a_ps[:, :nt], lhsT=g_ap, rhs=inv[:, :nt],
                                 start=True, stop=True)
                y1 = sbt.tile([H, NT], F32, tag="h0")
                nc.vector.tensor_mul(y1[:, :nt], oc[:, :nt], a_ps[:, :nt])
                if resid_ap is not None:
                    dst = out_ap
                    ret = None
                    if dst is None:
                        y = sbt.tile([H, NT], F32R, tag="yout")
                        dst = y[:, :nt]
                        ret = y
                    nc.vector.scalar_tensor_tensor(
                        out=dst, in0=y1[:, :nt], scalar=beta_ap, in1=resid_ap,
                        op0=ALU.add, op1=ALU.add)
                    return ret
                else:
                    dst = out_ap
                    ret = None
                    if dst is None:
                        y = sbt.tile([H, NT], F32R, tag="yout")
                        dst = y[:, :nt]
                        ret = y
                    nc.vector.tensor_scalar_add(dst, y1[:, :nt], beta_ap)
                    return ret

            # ---------------- encoders ----------------
            for t0 in range(0, xw, NT):
                nt = min(NT, xw - t0)
                na = sbt.tile([IN_NODE, NT], F32R, tag="attr")
                nc.sync.dma_start(out=na[:, :nt], in_=nattr[:, t0:t0 + nt])
                mlp_tile(nt, [(w0n_sb[:, :], na[:, :nt])],
                         wenc[:, 0:H], wenc[:, H:2 * H],
                         vecenc[:, 0:1], vecenc[:, 1:2], vecenc[:, 2:3],
                         vecenc[:, 3:4], genc[:, 0:H],
                         AF.Tanh, None, x_ext[:, t0:t0 + nt])
            for k in range(7):
                for t0 in range(0, widths[k], NT):
                    nt = min(NT, widths[k] - t0)
                    ea = sbt.tile([IN_EDGE, NT], F32R, tag="attr")
                    nc.sync.dma_start(out=ea[:, :nt],
                                      in_=eattr[:, eo[k] + t0:eo[k] + t0 + nt])
                    ye = mlp_tile(nt, [(w0e_sb[:, :], ea[:, :nt])],
                                  wenc[:, 2 * H:3 * H], wenc[:, 3 * H:4 * H],
                                  vecenc[:, 4:5], vecenc[:, 5:6], vecenc[:, 6:7],
                                  vecenc[:, 7:8], genc[:, H:2 * H],
                                  AF.Tanh, None, None)
                    nc.sync.dma_start(out=e_loc[:, eo[k] + t0:eo[k] + t0 + nt],
                                      in_=ye[:, :nt])

            # ---------------- message passing ----------------
            for it in range(n_iters):
                last = (it == n_iters - 1)
                wm = wpool.tile([H, 9 * H], F32R, tag="wm")
                nc.sync.dma_start(out=wm, in_=wmats[:, 9 * it * H:(9 * it + 9) * H])
                vv = wpool.tile([H, 8], F32, tag="vv")
                nc.sync.dma_start(out=vv, in_=vecs[:, 8 * it:8 * it + 8])
                gg = wpool.tile([1, 256], F32R, tag="gg")
                nc.sync.dma_start(out=gg, in_=gvecs[:, 256 * it:256 * it + 256])

                for k in range(7):
                    o = offs[k]
                    lo = 0 if last else -max(o, 0)
                    hi = M if last else lo + widths[k]
                    t0 = lo
                    while t0 < hi:
                        nt = min(NT, hi - t0)
                        ec = eo[k] + t0 + max(o, 0)
                        ein = sbt.tile([H, NT], F32R, tag="ein")
                        nc.sync.dma_start(out=ein[:, :nt], in_=e_loc[:, ec:ec + nt])
                        xa = x_ext[:, t0 + halo:t0 + halo + nt]
                        xb = x_ext[:, t0 + o + halo:t0 + o + halo + nt]
                        y = mlp_tile(
                            nt,
                            [(wm[:, 0:H], xa), (wm[:, H:2 * H], xb),
                             (wm[:, 2 * H:3 * H], ein[:, :nt])],
                            wm[:, 3 * H:4 * H], wm[:, 4 * H:5 * H],
                            vv[:, 0:1], vv[:, 1:2], vv[:, 2:3], vv[:, 3:4],
                            gg[:, 0:H], AF.Relu, ein[:, :nt], None)
                        if not last:
                            nc.sync.dma_start(out=e_loc[:, ec:ec + nt],
                                              in_=y[:, :nt])
                            aa = max(t0 + o, 0)
                            bb = min(t0 + nt + o, M)
                            if bb > aa:
                                ys = y[:, aa - (t0 + o):bb - (t0 + o)]
                                if k == 0:
                                    nc.vector.tensor_copy(out=agg[:, aa:bb], in_=ys)
                                else:
                                    nc.vector.tensor_add(agg[:, aa:bb],
                                                         agg[:, aa:bb], ys)
                        else:
                            d0_ps = psA.tile([H, NT], F32, tag="psA")
                            mm_acc(d0_ps, [(wenc[:, 4 * H:5 * H], y[:, :nt])], nt)
                            d0 = sbt.tile([H, NT], F32R, tag="h0")
                            nc.scalar.activation(out=d0[:, :nt], in_=d0_ps[:, :nt],
                                                 func=AF.Tanh, bias=vecenc[:, 8:9])
                            d1_ps = psB.tile([H, NT], F32, tag="psB")
                            mm_acc(d1_ps, [(wenc[:, 5 * H:6 * H], d0[:, :nt])], nt)
                            d1 = sbt.tile([H, NT], F32R, tag="h1")
                            nc.scalar.activation(out=d1[:, :nt], in_=d1_ps[:, :nt],
                                                 func=AF.Tanh, bias=vecenc[:, 9:10])
                            l_ps = psR.tile([1, NT], F32, tag="psR")
                            nc.tensor.matmul(out=l_ps[:, :nt], lhsT=wd2_sb[:, :],
                                             rhs=d1[:, :nt], start=True, stop=True)
                            l_sb = rows.tile([1, NT], F32, tag="lsb")
                            nc.scalar.activation(out=l_sb[:, :nt], in_=l_ps[:, :nt],
                                                 func=AF.Identity,
                                                 bias=vecenc[0:1, 10:11])
                            nc.sync.dma_start(out=l_out[k:k + 1, t0:t0 + nt],
                                              in_=l_sb[:, :nt])
                        t0 += nt

                if last:
                    break

                for t0 in range(0, M, NT):
                    nt = min(NT, M - t0)
                    xo = x_ext[:, t0 + halo:t0 + halo + nt]
                    mlp_tile(nt,
                             [(wm[:, 5 * H:6 * H], xo),
                              (wm[:, 6 * H:7 * H], agg[:, t0:t0 + nt])],
                             wm[:, 7 * H:8 * H], wm[:, 8 * H:9 * H],
                             vv[:, 4:5], vv[:, 5:6], vv[:, 6:7], vv[:, 7:8],
                             gg[:, H:2 * H], AF.Relu, xo, xo)

                ag_out = dram.tile([NCORES * H, 2 * halo], F32R,
                                   addr_space="Shared", tag=f"ag_out_{it}")
                nc.sync.dma_start(out=ag_in[:, 0:halo], in_=x_ext[:, halo:2 * halo])
                nc.sync.dma_start(out=ag_in[:, halo:2 * halo],
                                  in_=x_ext[:, M:M + halo])
                nc.gpsimd.collective_compute(
                    "AllGather", ALU.bypass,
                    replica_groups=[list(range(NCORES))],
                    ins=[ag_in[:].opt()], outs=[ag_out[:].opt()])
                nc.gpsimd.dma_start(out=x_ext[:, 0:halo],
                                    in_=ag_out[bass.ds(offL, H), halo:2 * halo])
                nc.gpsimd.dma_start(out=x_ext[:, M + halo:M + 2 * halo],
                                    in_=ag_out[bass.ds(offR, H), 0:halo])
    nc.finalize()
    return nc


# ======================= host-side driver =======================

def _center_w2(W2, b2):
    W2 = np.asarray(W2, np.float32)
    b2 = np.asarray(b2, np.float32)
    return W2 - W2.mean(axis=1, keepdims=True), b2 - b2.mean()


def kernel(node_attr, edge_attr, edge_index, input_x, input_r, rev_perm,
           selfloop_pos, params):
    node_attr = np.asarray(node_attr, np.float32)
    edge_attr = np.asarray(edge_attr, np.float32)
    edge_index = np.asarray(edge_index, np.int32)
    input_x = np.asarray(input_x, np.float32)
    rev_perm = np.asarray(rev_perm, np.int32)
    selfloop_pos = np.asarray(selfloop_pos, np.int32)

    src = edge_index[0].astype(np.int64)
    dst = edge_index[1].astype(np.int64)
    n = N
    delta = (dst - src) % n

    orig_offs = [0, 1, n - 1, 97, n - 97, 5001, n - 5001]
    offs = [_signed(CMUL * o) for o in orig_offs]
    tvals = np.full(delta.shape, -1, np.int64)
    for k in range(7):
        tvals[delta == (orig_offs[k] % n)] = k
    assert (tvals >= 0).all(), "unexpected graph structure"
    pos = (CMUL * src) % n
    ce = np.empty((7, n), np.int64)
    ce[tvals, pos] = np.arange(len(src))

    halo, xw, eo, widths, ew = _plan(offs)
    cinv = pow(CMUL, -1, n)

    key = (tuple(offs), N_ITERS)
    if key not in _NC_CACHE:
        _NC_CACHE[key] = build_nc(offs, N_ITERS)
    nc = _NC_CACHE[key]

    p = params

    def f32(a):
        return np.asarray(a, np.float32)

    mats, vcols, gcols = [], [], []
    for i in range(N_ITERS):
        ew2c, ebt2 = _center_w2(p['mp_edge']['W2'][i], p['mp_edge']['b2'][i])
        nw2c, nbt2 = _center_w2(p['mp_node']['W2'][i], p['mp_node']['b2'][i])
        W0e = f32(p['mp_edge']['W0'][i])
        mats += [W0e[0:H], W0e[H:2 * H], W0e[2 * H:3 * H],
                 f32(p['mp_edge']['W1'][i]), ew2c]
        W0nn = f32(p['mp_node']['W0'][i])
        mats += [W0nn[0:H], W0nn[H:2 * H], f32(p['mp_node']['W1'][i]), nw2c]
        vcols += [f32(p['mp_edge']['b0'][i]), f32(p['mp_edge']['b1'][i]), ebt2,
                  f32(p['mp_edge']['beta'][i]),
                  f32(p['mp_node']['b0'][i]), f32(p['mp_node']['b1'][i]), nbt2,
                  f32(p['mp_node']['beta'][i])]
        gcols += [f32(p['mp_edge']['g'][i]), f32(p['mp_node']['g'][i])]
    new2c, nebt2 = _center_w2(p['node_enc']['W2'], p['node_enc']['b2'])
    eew2c, eebt2 = _center_w2(p['edge_enc']['W2'], p['edge_enc']['b2'])
    mats += [f32(p['node_enc']['W1']), new2c, f32(p['edge_enc']['W1']), eew2c,
             f32(p['dec_L']['W0']), f32(p['dec_L']['W1'])]
    b2d = np.zeros(H, np.float32)
    b2d[0] = f32(p['dec_L']['b2']).reshape(-1)[0]
    vcols += [f32(p['node_enc']['b0']), f32(p['node_enc']['b1']), nebt2,
              f32(p['node_enc']['beta']),
              f32(p['edge_enc']['b0']), f32(p['edge_enc']['b1']), eebt2,
              f32(p['edge_enc']['beta']),
              f32(p['dec_L']['b0']), f32(p['dec_L']['b1']), b2d,
              np.full(H, EPS, np.float32)]
    gcols += [f32(p['node_enc']['g']), f32(p['edge_enc']['g'])]

    wmats_np = np.ascontiguousarray(np.concatenate(mats, axis=1), dtype=np.float32)
    vecs_np = np.ascontiguousarray(np.stack(vcols, axis=1), dtype=np.float32)
    gvecs_np = np.ascontiguousarray(np.concatenate(gcols)[None, :], dtype=np.float32)
    w0n_np = np.ascontiguousarray(f32(p['node_enc']['W0']))
    w0e_np = np.ascontiguousarray(f32(p['edge_enc']['W0']))
    wd2_np = np.ascontiguousarray(f32(p['dec_L']['W2']))

    new_ids = (cinv * np.arange(n, dtype=np.int64)) % n
    can_na = node_attr[new_ids]            # [n, 6]
    can_ea = edge_attr[ce]                 # [7, n, 4]

    in_maps = []
    for c in range(NCORES):
        start = c * M
        xcols = np.arange(start - halo, start - halo + xw, dtype=np.int64) % n
        nat = np.ascontiguousarray(can_na[xcols].T)
        eat = np.empty((IN_EDGE, ew), np.float32)
        for k in range(7):
            o = offs[k]
            lo = start - max(o, 0)
            cols = np.arange(lo, lo + widths[k], dtype=np.int64) % n
            eat[:, eo[k]:eo[k] + widths[k]] = can_ea[k][cols].T
        hoff = np.array([[((c - 1) % NCORES) * H, ((c + 1) % NCORES) * H]],
                        np.uint32)
        in_maps.append({
            "nattr": nat, "eattr": eat, "wmats": wmats_np, "vecs": vecs_np,
            "gvecs": gvecs_np, "w0n": w0n_np, "w0e": w0e_np, "wd2": wd2_np,
            "onesc": np.ones((H, 1), np.float32),
            "halo_off": hoff,
        })

    res = run_bass_kernel_spmd(nc, in_maps, core_ids=list(range(NCORES)))
    l_can = np.concatenate([res.results[c]["L_out"] for c in range(NCORES)],
                           axis=1)  # [7, n]

    L = np.empty(len(src), np.float32)
    L[ce] = l_can

    L = 0.5 * (L + L[rev_perm])
    diag_full = edge_attr[:, -1] + edge_attr[:, -2]
    L = np.where(src < dst, 0.0, L)
    L = np.where(src == dst, np.sqrt(diag_full), L).astype(np.float32)
    diag_ele = diag_full[selfloop_pos][:, None].astype(np.float32)

    xflat = input_x[:, 0].astype(np.float64)
    Ld = L.astype(np.float64)
    LTx = np.bincount(dst, weights=Ld * xflat[src], minlength=n)
    LLTx = np.bincount(src, weights=Ld * LTx[dst], minlength=n)
    dirichlet = node_attr[:, 3] != 0
    b = np.where(dirichlet, xflat, LLTx).astype(np.float32)[:, None]

    output_x = np.zeros_like(input_x, dtype=np.float32)
    return (b, ((L[:, None], diag_ele, 1.0), edge_index), output_x)
